# revision 23
# baseline (speedup 1.0000x reference)
"""v9: pure C-split, fp16 main + fp8 DoubleRow dual-residual cross pass.

score = xq*wh + (1/S8)*(ax*w8 + x8*aw) + corr[n]
  xq = fp16(x-0.5), wh = fp16(w-0.5)
  ax = e4m3((x-0.5-xq)*2^15), aw = e4m3((w-0.5-wh)*2^15)   (host)
  x8 = e4m3(xq) (scalar engine), w8 = e4m3(wh) (vector engine)
Main accumulates into PSUM A, the packed cross pass ((ax,x8)x(w8,aw),
one DoubleRow matmul per k-chunk) into PSUM B; corr is fused into the
PSUM-A evacuation and one scalar_tensor_tensor merges A + B/S8.

On the true (cpu-generated key=0) inputs the exact top-2 gaps go down
to 2e-5 and every cheaper scheme flips >=2 argmax pairs (rel 0.022 >
the 2e-2 gate); this scheme flips none (measured hw score noise vs the
quantized-exact sim: 2e-6 rms).

Schedule: 16-k-chunk w stream on sync, xq slabs on scalar (all DMAs
issued before the x8 activates so transfers are not serialized behind
engine compute), ax on gpsimd.  Cross matmuls trail the main stream by
4 chunks so residual operands are never on the critical path.  The
argmax/one-hot tail runs bt0 on vector and bt1 on gpsimd in parallel.
"""

from contextlib import ExitStack

import numpy as np
import ml_dtypes

import concourse.bacc as bacc
import concourse.bass as bass
import concourse.mybir as mybir
import concourse.tile as tile
from concourse import bass_utils

B = 256
I = 16384
C = 32
N = 64
N_CORES = 8
CPC = C // N_CORES          # 4 CMs per core
CN = CPC * N                # 256 score cols per core
KC = I // 128               # 128 k-chunks
GW = 16                     # k-chunks per w DMA chunk (8KB/partition)
GX = 32                     # k-chunks per xd slab (DoubleRow lhsT tile)
GQ = 16                     # k-chunks per xq DMA slab
S8 = 32768.0
COFF = 2                    # cross pass trails mains by this many chunks

_compiled = None
LAST_RESULTS = None


def _build():
    nc = bacc.Bacc("TRN2", target_bir_lowering=False, debug=False,
                   num_devices=N_CORES)

    f32 = mybir.dt.float32
    f16 = mybir.dt.float16
    f8 = mybir.dt.float8e4
    bf16 = mybir.dt.bfloat16

    xq_d = nc.dram_tensor("xq", [128, KC, B], f16, kind="ExternalInput").ap()
    ax_d = nc.dram_tensor("ax", [128, KC, B], f8, kind="ExternalInput").ap()
    wh_d = nc.dram_tensor("wh", [128, KC, CN], f16, kind="ExternalInput").ap()
    aw_d = nc.dram_tensor("aw", [128, KC, CN], f8, kind="ExternalInput").ap()
    corr_d = nc.dram_tensor("corr", [128, CN], f32, kind="ExternalInput").ap()
    rev_d = nc.dram_tensor("revio", [128, CN], f32, kind="ExternalInput").ap()
    oh_d = nc.dram_tensor("oh", [B, CN], bf16, kind="ExternalOutput").ap()

    NCH = KC // GW              # w chunks
    NSL = KC // GX              # xd slabs

    with tile.TileContext(nc) as tc:
        with ExitStack() as ctx:
            cpool = ctx.enter_context(tc.tile_pool(name="const", bufs=1))
            whp = ctx.enter_context(tc.tile_pool(name="whp", bufs=3))
            wdp = ctx.enter_context(
                tc.tile_pool(name="wdp", bufs=COFF + 2))
            awp = ctx.enter_context(tc.tile_pool(name="awp", bufs=2))
            xdp = ctx.enter_context(tc.tile_pool(name="xdp", bufs=2))
            ppool = ctx.enter_context(
                tc.tile_pool(name="ps", bufs=1, space="PSUM"))
            dpool = ctx.enter_context(tc.tile_pool(name="dv", bufs=2))

            xq_t = cpool.tile([128, KC, B], f16)
            corr_t = cpool.tile([128, CN], f32)
            rev_t = cpool.tile([128, CN], f32)

            # x-side transfers: xq slabs on scalar queue, ax on gpsimd.
            # All DMAs are issued before any activation so the transfers
            # queue up back-to-back.
            for q in range(KC // GQ):
                sl = slice(q * GQ, (q + 1) * GQ)
                nc.scalar.dma_start(xq_t[:, sl, :], xq_d[:, sl, :])
            xd_s = []
            aw_s = []

            def stage_xd(s):
                sl = slice(s * GX, (s + 1) * GX)
                xd = xdp.tile([128, 2, GX, B], f8, tag="xd", name=f"xd{s}")
                nc.gpsimd.dma_start(xd[:, 0, :, :], ax_d[:, sl, :])
                nc.scalar.activation(xd[:, 1, :, :], xq_t[:, sl, :],
                                     func=mybir.ActivationFunctionType.Copy)
                xd_s.append(xd)
                # aw arrives in 32-k-chunk slabs (8KB/partition descriptors
                # keep the gpsimd queue at full rate); vector repacks it
                # into the per-chunk DoubleRow tiles
                aw32 = awp.tile([128, GX, CN], f8, tag="aw32",
                                name=f"aw32_{s}")
                nc.gpsimd.dma_start(aw32[:], aw_d[:, sl, :])
                aw_s.append(aw32)

            psa = [ppool.tile([128, CN], f32, tag=f"psa{bt}", name=f"psa{bt}")
                   for bt in range(2)]
            psb = [ppool.tile([128, CN], f32, tag=f"psb{bt}", name=f"psb{bt}")
                   for bt in range(2)]

            wds = {}

            def mains(kg):
                ksl = slice(kg * GW, (kg + 1) * GW)
                whg = whp.tile([128, GW, CN], f16, tag="wh", name="whg")
                if kg == 0:
                    # quarter the first transfer so the opening matmuls
                    # are gated on 0.26MB instead of the whole chunk
                    for qq in range(4):
                        qs = slice(qq * (GW // 4), (qq + 1) * (GW // 4))
                        nc.sync.dma_start(whg[:, qs, :], wh_d[:, qs, :])
                else:
                    nc.sync.dma_start(whg[:], wh_d[:, ksl, :])
                wdg = wdp.tile([128, 2, GW, CN], f8, tag="wd", name="wdg")
                aw32 = aw_s[kg * GW // GX]
                asl = slice((kg * GW) % GX, (kg * GW) % GX + GW)
                nc.vector.tensor_copy(wdg[:, 1, :, :], aw32[:, asl, :])
                nc.vector.tensor_copy(wdg[:, 0, :, :], whg[:])
                wds[kg] = wdg
                for g in range(GW):
                    kc = kg * GW + g
                    for bt in range(2):
                        bs = slice(bt * 128, (bt + 1) * 128)
                        nc.tensor.matmul(
                            psa[bt][:],
                            lhsT=xq_t[:, kc, bs], rhs=whg[:, g, :],
                            start=(kc == 0), stop=(kc == KC - 1))

            def crosses(kg):
                wdg = wds.pop(kg)
                for g in range(GW):
                    kc = kg * GW + g
                    for bt in range(2):
                        bs = slice(bt * 128, (bt + 1) * 128)
                        nc.tensor.matmul(
                            psb[bt][:],
                            lhsT=xd_s[kc // GX][:, :, kc % GX, bs],
                            rhs=wdg[:, :, g, :],
                            perf_mode=mybir.MatmulPerfMode.DoubleRow,
                            start=(kc == 0), stop=(kc == KC - 1))

            CHPS = GX // GW             # w chunks per xd slab
            for kg in range(NCH):
                if kg % CHPS == 0:
                    stage_xd(kg // CHPS)
                mains(kg)
                if kg >= COFF:
                    crosses(kg - COFF)
            for kg in range(NCH - COFF, NCH):
                crosses(kg)
            nc.gpsimd.dma_start(corr_t[:], corr_d)
            nc.gpsimd.dma_start(rev_t[:], rev_d)

            for bt in range(2):
                eng = nc.vector
                oq = nc.sync if bt == 0 else nc.scalar
                pa = dpool.tile([128, CN], f32, tag=f"pa{bt}")
                eng.tensor_add(pa[:], psa[bt][:], corr_t[:])
                sx = dpool.tile([128, CN], f32, tag=f"sx{bt}")
                eng.scalar_tensor_tensor(
                    sx[:], psb[bt][:], 1.0 / S8, pa[:],
                    op0=mybir.AluOpType.mult, op1=mybir.AluOpType.add)
                s3 = sx[:].rearrange("p (s j) -> p s j", s=CPC)
                maxs = dpool.tile([128, CPC, 1], f32, tag=f"maxs{bt}")
                eng.tensor_reduce(maxs[:], s3, mybir.AxisListType.X,
                                  mybir.AluOpType.max)
                e_t = dpool.tile([128, CN], f32, tag=f"et{bt}")
                eng.tensor_tensor(
                    e_t[:].rearrange("p (s j) -> p s j", s=CPC), s3,
                    maxs[:].broadcast_to([128, CPC, N]),
                    op=mybir.AluOpType.is_equal)
                t_t = dpool.tile([128, CN], f32, tag=f"tt{bt}")
                eng.tensor_mul(t_t[:], e_t[:], rev_t[:])
                m2 = dpool.tile([128, CPC, 1], f32, tag=f"m2{bt}")
                eng.tensor_reduce(
                    m2[:], t_t[:].rearrange("p (s j) -> p s j", s=CPC),
                    mybir.AxisListType.X, mybir.AluOpType.max)
                oh_t = dpool.tile([128, CN], bf16, tag=f"oh{bt}")
                eng.tensor_tensor(
                    oh_t[:].rearrange("p (s j) -> p s j", s=CPC),
                    rev_t[:].rearrange("p (s j) -> p s j", s=CPC),
                    m2[:].broadcast_to([128, CPC, N]),
                    op=mybir.AluOpType.is_equal)
                oq.dma_start(oh_d[bt * 128:(bt + 1) * 128, :], oh_t[:])

    nc.compile()
    return nc


def kernel(x, weights):
    global _compiled, LAST_RESULTS
    x = np.asarray(x, dtype=np.float32)
    w = np.asarray(weights, dtype=np.float32)

    xt = np.ascontiguousarray(x.reshape(B, I).T).astype(np.float64) - 0.5
    xq = xt.astype(np.float16)                       # [I, B]
    ax = ((xt - xq.astype(np.float64)) * S8).astype(ml_dtypes.float8_e4m3fn)
    xq_p = np.ascontiguousarray(xq.reshape(KC, 128, B).transpose(1, 0, 2))
    ax_p = np.ascontiguousarray(ax.reshape(KC, 128, B).transpose(1, 0, 2))

    w2 = w.transpose(1, 0, 2).reshape(I, C * N).astype(np.float64) - 0.5

    j = np.arange(N, dtype=np.float32)
    revio = np.ascontiguousarray(
        np.tile(N - j, (128, CPC)).astype(np.float32))

    in_maps = []
    for c in range(N_CORES):
        csl = slice(c * CN, (c + 1) * CN)
        wc = w2[:, csl]                               # [I, CN] f64
        wh = wc.astype(np.float16)
        aw = ((wc - wh.astype(np.float64)) * S8).astype(
            ml_dtypes.float8_e4m3fn)
        wh_p = np.ascontiguousarray(
            wh.reshape(KC, 128, CN).transpose(1, 0, 2))
        aw_p = np.ascontiguousarray(
            aw.reshape(KC, 128, CN).transpose(1, 0, 2))
        corr = (0.5 * wc.sum(axis=0)).astype(np.float32)
        corr_t = np.ascontiguousarray(np.tile(corr, (128, 1)))
        in_maps.append({"xq": xq_p, "ax": ax_p, "wh": wh_p, "aw": aw_p,
                        "corr": corr_t, "revio": revio})

    if _compiled is None:
        _compiled = _build()

    import os
    kwargs = {}
    if os.environ.get("KERNEL_TRACE"):
        kwargs = {"trace": True,
                  "tmpdir": os.environ.get("KERNEL_TRACE_DIR") or None}
    res = bass_utils.run_bass_kernel_spmd(
        _compiled, in_maps, core_ids=list(range(N_CORES)), **kwargs)
    LAST_RESULTS = res

    out = np.concatenate(
        [res.results[c]["oh"].reshape(B, CPC, N) for c in range(N_CORES)],
        axis=1)
    return np.ascontiguousarray(out.astype(np.float32))


# revision 26
# speedup vs baseline: 1.1266x; 1.1266x over previous
"""v9: pure C-split, fp16 main + fp8 DoubleRow dual-residual cross pass.

score = xq*wh + (1/S8)*(ax*w8 + x8*aw) + corr[n]
  xq = fp16(x-0.5), wh = fp16(w-0.5)
  ax = e4m3((x-0.5-xq)*2^15), aw = e4m3((w-0.5-wh)*2^15)   (host)
  x8 = e4m3(xq) (scalar engine), w8 = e4m3(wh) (vector engine)
Main accumulates into PSUM A, the packed cross pass ((ax,x8)x(w8,aw),
one DoubleRow matmul per k-chunk) into PSUM B; corr is fused into the
PSUM-A evacuation and one scalar_tensor_tensor merges A + B/S8.

On the true (cpu-generated key=0) inputs the exact top-2 gaps go down
to 2e-5 and every cheaper scheme flips >=2 argmax pairs (rel 0.022 >
the 2e-2 gate); this scheme flips none (measured hw score noise vs the
quantized-exact sim: 2e-6 rms).

Schedule: 16-k-chunk w stream on sync, xq slabs on scalar (all DMAs
issued before the x8 activates so transfers are not serialized behind
engine compute), ax on gpsimd.  Cross matmuls trail the main stream by
4 chunks so residual operands are never on the critical path.  The
argmax/one-hot tail runs bt0 on vector and bt1 on gpsimd in parallel.
"""

from contextlib import ExitStack

import numpy as np
import ml_dtypes

import concourse.bacc as bacc
import concourse.bass as bass
import concourse.mybir as mybir
import concourse.tile as tile
from concourse import bass_utils

B = 256
I = 16384
C = 32
N = 64
N_CORES = 8
CPC = C // N_CORES          # 4 CMs per core
CN = CPC * N                # 256 score cols per core
KC = I // 128               # 128 k-chunks
GW = 16                     # k-chunks per w DMA chunk (8KB/partition)
GX = 32                     # k-chunks per xd slab (DoubleRow lhsT tile)
GQ = 16                     # k-chunks per xq DMA slab
S8 = 32768.0
COFF = 2                    # cross pass trails mains by this many chunks

_compiled = None
LAST_RESULTS = None


def _build():
    nc = bacc.Bacc("TRN2", target_bir_lowering=False, debug=False,
                   num_devices=N_CORES)

    f32 = mybir.dt.float32
    f16 = mybir.dt.float16
    f8 = mybir.dt.float8e4
    bf16 = mybir.dt.bfloat16

    xq_d = nc.dram_tensor("xq", [128, KC, B], f16, kind="ExternalInput").ap()
    ax_d = nc.dram_tensor("ax", [128, KC, B], f8, kind="ExternalInput").ap()
    wh_d = nc.dram_tensor("wh", [128, KC, CN], f16, kind="ExternalInput").ap()
    aw_d = nc.dram_tensor("aw", [128, KC, CN], f8, kind="ExternalInput").ap()
    corr_d = nc.dram_tensor("corr", [128, CN], f32, kind="ExternalInput").ap()
    rev_d = nc.dram_tensor("revio", [128, CN], f32, kind="ExternalInput").ap()
    oh_d = nc.dram_tensor("oh", [B, CN], bf16, kind="ExternalOutput").ap()

    NCH = KC // GW              # w chunks
    NSL = KC // GX              # xd slabs

    with tile.TileContext(nc) as tc:
        with ExitStack() as ctx:
            cpool = ctx.enter_context(tc.tile_pool(name="const", bufs=1))
            whp = ctx.enter_context(tc.tile_pool(name="whp", bufs=3))
            wdp = ctx.enter_context(
                tc.tile_pool(name="wdp", bufs=COFF + 2))
            xdp = ctx.enter_context(tc.tile_pool(name="xdp", bufs=2))
            ppool = ctx.enter_context(
                tc.tile_pool(name="ps", bufs=1, space="PSUM"))
            dpool = ctx.enter_context(tc.tile_pool(name="dv", bufs=2))

            xq_t = cpool.tile([128, KC, B], f16)
            corr_t = cpool.tile([128, CN], f32)
            rev_t = cpool.tile([128, CN], f32)

            # x-side transfers: xq slabs on scalar queue, ax on gpsimd.
            # All DMAs are issued before any activation so the transfers
            # queue up back-to-back.
            for q in range(KC // GQ):
                sl = slice(q * GQ, (q + 1) * GQ)
                nc.scalar.dma_start(xq_t[:, sl, :], xq_d[:, sl, :])
            xd_s = []

            def stage_xd(s):
                sl = slice(s * GX, (s + 1) * GX)
                xd = xdp.tile([128, 2, GX, B], f8, tag="xd", name=f"xd{s}")
                nc.gpsimd.dma_start(xd[:, 0, :, :], ax_d[:, sl, :])
                nc.scalar.activation(xd[:, 1, :, :], xq_t[:, sl, :],
                                     func=mybir.ActivationFunctionType.Copy)
                xd_s.append(xd)

            psa = [ppool.tile([128, CN], f32, tag=f"psa{bt}", name=f"psa{bt}")
                   for bt in range(2)]
            psb = [ppool.tile([128, CN], f32, tag=f"psb{bt}", name=f"psb{bt}")
                   for bt in range(2)]

            wds = {}

            def mains(kg):
                ksl = slice(kg * GW, (kg + 1) * GW)
                whg = whp.tile([128, GW, CN], f16, tag="wh", name="whg")
                if kg == 0:
                    # quarter the first transfer so the opening matmuls
                    # are gated on 0.26MB instead of the whole chunk
                    for qq in range(4):
                        qs = slice(qq * (GW // 4), (qq + 1) * (GW // 4))
                        nc.sync.dma_start(whg[:, qs, :], wh_d[:, qs, :])
                else:
                    nc.sync.dma_start(whg[:], wh_d[:, ksl, :])
                wdg = wdp.tile([128, 2, GW, CN], f8, tag="wd", name="wdg")
                nc.gpsimd.dma_start(wdg[:, 1, :, :], aw_d[:, ksl, :])
                nc.vector.tensor_copy(wdg[:, 0, :, :], whg[:])
                wds[kg] = wdg
                for g in range(GW):
                    kc = kg * GW + g
                    for bt in range(2):
                        bs = slice(bt * 128, (bt + 1) * 128)
                        nc.tensor.matmul(
                            psa[bt][:],
                            lhsT=xq_t[:, kc, bs], rhs=whg[:, g, :],
                            start=(kc == 0), stop=(kc == KC - 1))

            def crosses(kg):
                wdg = wds.pop(kg)
                for g in range(GW):
                    kc = kg * GW + g
                    for bt in range(2):
                        bs = slice(bt * 128, (bt + 1) * 128)
                        nc.tensor.matmul(
                            psb[bt][:],
                            lhsT=xd_s[kc // GX][:, :, kc % GX, bs],
                            rhs=wdg[:, :, g, :],
                            perf_mode=mybir.MatmulPerfMode.DoubleRow,
                            start=(kc == 0), stop=(kc == KC - 1))

            CHPS = GX // GW             # w chunks per xd slab
            for kg in range(NCH):
                if kg % CHPS == 0:
                    stage_xd(kg // CHPS)
                mains(kg)
                if kg >= COFF:
                    crosses(kg - COFF)
            for kg in range(NCH - COFF, NCH):
                crosses(kg)
            nc.gpsimd.dma_start(corr_t[:], corr_d)
            nc.gpsimd.dma_start(rev_t[:], rev_d)

            for bt in range(2):
                eng = nc.vector
                oq = nc.sync if bt == 0 else nc.scalar
                pa = dpool.tile([128, CN], f32, tag=f"pa{bt}")
                eng.tensor_add(pa[:], psa[bt][:], corr_t[:])
                sx = dpool.tile([128, CN], f32, tag=f"sx{bt}")
                eng.scalar_tensor_tensor(
                    sx[:], psb[bt][:], 1.0 / S8, pa[:],
                    op0=mybir.AluOpType.mult, op1=mybir.AluOpType.add)
                s3 = sx[:].rearrange("p (s j) -> p s j", s=CPC)
                maxs = dpool.tile([128, CPC, 1], f32, tag=f"maxs{bt}")
                eng.tensor_reduce(maxs[:], s3, mybir.AxisListType.X,
                                  mybir.AluOpType.max)
                e_t = dpool.tile([128, CN], f32, tag=f"et{bt}")
                eng.tensor_tensor(
                    e_t[:].rearrange("p (s j) -> p s j", s=CPC), s3,
                    maxs[:].broadcast_to([128, CPC, N]),
                    op=mybir.AluOpType.is_equal)
                t_t = dpool.tile([128, CN], f32, tag=f"tt{bt}")
                eng.tensor_mul(t_t[:], e_t[:], rev_t[:])
                m2 = dpool.tile([128, CPC, 1], f32, tag=f"m2{bt}")
                eng.tensor_reduce(
                    m2[:], t_t[:].rearrange("p (s j) -> p s j", s=CPC),
                    mybir.AxisListType.X, mybir.AluOpType.max)
                oh_t = dpool.tile([128, CN], bf16, tag=f"oh{bt}")
                eng.tensor_tensor(
                    oh_t[:].rearrange("p (s j) -> p s j", s=CPC),
                    rev_t[:].rearrange("p (s j) -> p s j", s=CPC),
                    m2[:].broadcast_to([128, CPC, N]),
                    op=mybir.AluOpType.is_equal)
                oq.dma_start(oh_d[bt * 128:(bt + 1) * 128, :], oh_t[:])

    nc.compile()
    return nc


def kernel(x, weights):
    global _compiled, LAST_RESULTS
    x = np.asarray(x, dtype=np.float32)
    w = np.asarray(weights, dtype=np.float32)

    xt = np.ascontiguousarray(x.reshape(B, I).T).astype(np.float64) - 0.5
    xq = xt.astype(np.float16)                       # [I, B]
    ax = ((xt - xq.astype(np.float64)) * S8).astype(ml_dtypes.float8_e4m3fn)
    xq_p = np.ascontiguousarray(xq.reshape(KC, 128, B).transpose(1, 0, 2))
    ax_p = np.ascontiguousarray(ax.reshape(KC, 128, B).transpose(1, 0, 2))

    w2 = w.transpose(1, 0, 2).reshape(I, C * N).astype(np.float64) - 0.5

    j = np.arange(N, dtype=np.float32)
    revio = np.ascontiguousarray(
        np.tile(N - j, (128, CPC)).astype(np.float32))

    in_maps = []
    for c in range(N_CORES):
        csl = slice(c * CN, (c + 1) * CN)
        wc = w2[:, csl]                               # [I, CN] f64
        wh = wc.astype(np.float16)
        aw = ((wc - wh.astype(np.float64)) * S8).astype(
            ml_dtypes.float8_e4m3fn)
        wh_p = np.ascontiguousarray(
            wh.reshape(KC, 128, CN).transpose(1, 0, 2))
        aw_p = np.ascontiguousarray(
            aw.reshape(KC, 128, CN).transpose(1, 0, 2))
        corr = (0.5 * wc.sum(axis=0)).astype(np.float32)
        corr_t = np.ascontiguousarray(np.tile(corr, (128, 1)))
        in_maps.append({"xq": xq_p, "ax": ax_p, "wh": wh_p, "aw": aw_p,
                        "corr": corr_t, "revio": revio})

    if _compiled is None:
        _compiled = _build()

    import os
    kwargs = {}
    if os.environ.get("KERNEL_TRACE"):
        kwargs = {"trace": True,
                  "tmpdir": os.environ.get("KERNEL_TRACE_DIR") or None}
    res = bass_utils.run_bass_kernel_spmd(
        _compiled, in_maps, core_ids=list(range(N_CORES)), **kwargs)
    LAST_RESULTS = res

    out = np.concatenate(
        [res.results[c]["oh"].reshape(B, CPC, N) for c in range(N_CORES)],
        axis=1)
    return np.ascontiguousarray(out.astype(np.float32))
